# revision 19
# baseline (speedup 1.0000x reference)
"""KAN 3x3 convolution (single KANLinear shared across channels) on 8 TRN2 cores.

Math: for x in [0,1) on the fixed uniform spline grid, every per-feature
function g_f(t) = spline_f(t) + base_w_f*silu(t) is smooth with (at most) C^2
breaks at the interior knots 0.2 / 0.6.  We fit (host-side, vs the actual
runtime weights) a truncated-power basis {1, t, t^2, t^3, relu(t-k_i)^3} with
an adaptively chosen knot set: [] -> 3 channels, [k*] -> 4, [0.2,0.6] -> 5
(exact).  The smallest basis whose predicted output error (measured on patches
sampled from the real input) is < TH is used, so the whole KAN conv collapses
to   out = bias + conv3x3(W, [x, x^2, x^3, relu(x-k_i)^3...]).

On-device per core: pointwise channel build (DVE/ACT, bf16) + NCH*3 banded
matmuls on TensorE (contraction over image rows; bands encode dy taps,
free-dim shifts encode dx), PSUM-accumulated per 4-image group, extracted
with the bias add fused into the ScalarE copy (bf16 out, SBUF-native layout;
the host undoes the group interleave).
"""

import numpy as np
import ml_dtypes

B, C, H, W = 16, 8, 128, 128
KERNEL = 3
HO = WO = H - KERNEL + 1  # 126
SPLINE_ORDER = 3
N_CORES = 8
IMG_PER_CORE = (B * C) // N_CORES  # 16
GROUP = 4                          # images per matmul group
N_GROUPS = IMG_PER_CORE // GROUP   # 4
BF16 = ml_dtypes.bfloat16
FIT_TH = 1.2e-2                    # predicted rel-err budget for basis choice

_NC_CACHE = {}


def _bsplines_np(t, grid):
    """Port of reference b_splines in numpy float64. t: (N,), grid: (F, G)."""
    F = grid.shape[0]
    x = np.tile(t[:, None], (1, F))[..., None]       # (N, F, 1)
    g = grid[None, :, :]                             # (1, F, G)
    bases = ((x >= g[:, :, :-1]) & (x < g[:, :, 1:])).astype(np.float64)
    for k in range(1, SPLINE_ORDER + 1):
        bases = ((x - g[:, :, : -(k + 1)]) / (g[:, :, k:-1] - g[:, :, : -(k + 1)])
                 * bases[:, :, :-1]
                 + (g[:, :, k + 1:] - x) / (g[:, :, k + 1:] - g[:, :, 1:-k])
                 * bases[:, :, 1:])
    return bases                                     # (N, F, G - order - 1)


def _host_fit(base_weight, spline_weight, spline_scaler, grid, x):
    """Pick the smallest truncated-power basis representing all 9 per-feature
    functions g_f within FIT_TH of the (sampled) output RMS.

    Returns (knots list, Wc [nch, 9] fp64 channel weights, bias fp64)."""
    c = (spline_weight[0].astype(np.float64)
         * spline_scaler[0].astype(np.float64)[:, None])          # (9, 8)
    t = np.linspace(0.0, 1.0, 4097, endpoint=False) + 1.0 / 8194.0
    bases = _bsplines_np(t, grid.astype(np.float64))              # (N, 9, 8)
    g = np.einsum("nfj,fj->nf", bases, c)                         # (N, 9)
    g += base_weight[0].astype(np.float64)[None, :] * (t / (1.0 + np.exp(-t)))[:, None]

    # output RMS estimate from patches of the actual input
    rng = np.random.RandomState(0)
    xf = x.reshape(B * C, H, W).astype(np.float64)
    ii = rng.randint(0, B * C, 20000)
    yy = rng.randint(0, HO, 20000)
    xx = rng.randint(0, WO, 20000)
    acc = np.zeros(20000)
    f = 0
    for dy in range(3):
        for dx in range(3):
            v = xf[ii, yy + dy, xx + dx]
            acc += np.interp(v, t, g[:, f])
            f += 1
    out_rms = max(np.sqrt((acc ** 2).mean()), 1e-30)

    def basis(knots):
        cols = [np.ones_like(t), t, t * t, t ** 3]
        cols += [np.maximum(t - k, 0.0) ** 3 for k in knots]
        return np.stack(cols, axis=-1)

    def fit(knots):
        A = basis(knots)
        coef, _, _, _ = np.linalg.lstsq(A, g, rcond=None)
        resid = A @ coef - g
        rel = np.sqrt((resid ** 2).mean(0).sum()) / out_rms
        return coef, rel

    cands = [[]]
    best_k, best_rel = None, None
    for k in np.linspace(0.05, 0.95, 91):
        _, rel = fit([k])
        if best_rel is None or rel < best_rel:
            best_k, best_rel = float(k), rel
    cands.append([best_k])
    cands.append([0.2, 0.6])
    for knots in cands:
        coef, rel = fit(knots)
        if rel < FIT_TH or knots == [0.2, 0.6]:
            Wc = coef[1:].copy()                  # (nch, 9): t, t^2, t^3, hinges
            bias = coef[0].sum()
            return knots, Wc, bias
    raise AssertionError("unreachable")


def _banded_lhsT(Wc):
    """[128, NCH*3*126] bf16: per (ch, dx) a banded [128,126] with
    W[ch, dy, dx] on diagonals (row i+dy, col i)."""
    nch = Wc.shape[0]
    Wk = Wc.reshape(nch, 3, 3)            # (ch, dy, dx)
    out = np.zeros((H, nch * 3 * HO), dtype=np.float64)
    ii = np.arange(HO)
    for ch in range(nch):
        for dx in range(3):
            tt = ch * 3 + dx
            for dy in range(3):
                out[ii + dy, tt * HO + ii] = Wk[ch, dy, dx]
    return out.astype(BF16)


def _build_nc(knots, bias):
    import concourse.bass as bass
    import concourse.mybir as mybir
    from concourse.tile import TileContext

    f32 = mybir.dt.float32
    bf16 = mybir.dt.bfloat16
    AF = mybir.ActivationFunctionType
    ALU = mybir.AluOpType
    nch = 3 + len(knots)
    nmm = nch * 3

    nc = bass.Bass()
    xs = nc.declare_dram_parameter("xs", [IMG_PER_CORE, H, W], f32, isOutput=False)
    wb = nc.declare_dram_parameter("wb", [H, nmm * HO], bf16, isOutput=False)
    out = nc.declare_dram_parameter(
        "out", [N_GROUPS, HO, GROUP, WO], bf16, isOutput=True)

    with TileContext(nc) as tc:
        with tc.tile_pool(name="wpool", bufs=1) as wpool, \
             tc.tile_pool(name="xpool", bufs=1) as xpool, \
             tc.tile_pool(name="zpool", bufs=2) as zpool, \
             tc.tile_pool(name="opool", bufs=1) as opool, \
             tc.tile_pool(name="cpool", bufs=1) as cpool, \
             tc.tile_pool(name="psum", bufs=4, space="PSUM") as pp, \
             tc.tile_pool(name="psumj", bufs=1, space="PSUM") as ppj:
            # --- PE warm-up: junk matmuls during the DMA fill keep the HAM
            # activity window busy so the real matmuls run at 2.4 GHz.  Any
            # PE idle gap resets the HAM busy streak, so the dummies must
            # hand off seamlessly to the first real matmul.
            junk = cpool.tile([H, 512], bf16, tag="junk")
            nc.gpsimd.memset(junk[:, :], 0.0)
            ptj = ppj.tile([H, 512], f32, tag="jacc")
            for _ in range(5):
                nc.tensor.matmul(ptj[:, :], junk[:, :H], junk[:, :],
                                 start=True, stop=True)
            for _ in range(9):
                nc.tensor.matmul(ptj[:, :128], junk[:, :H], junk[:, :128],
                                 start=True, stop=True)

            # --- input DMAs.  group-0 images go first and own the SDMA
            # engines (the bulk-x DMA is WAW-delayed behind them below),
            # split across the sync and vector DGE rings so their triggers
            # issue in parallel; weight bands ride the scalar-engine ring.
            # The scalar-engine DGE ring generates descriptors in FIFO order,
            # so queueing [xt0b, wtA, wtB, xt1, xt23] there delivers each
            # piece in priority order with per-piece completion semaphores,
            # while the sync ring carries xt0a in parallel.
            xt0 = xpool.tile([H, GROUP, W], f32, tag="x0")
            nc.sync.dma_start(
                out=xt0[:, :GROUP // 2, :],
                in_=xs[0:GROUP // 2].rearrange("g i j -> i g j"))
            nc.scalar.dma_start(
                out=xt0[:, GROUP // 2:, :],
                in_=xs[GROUP // 2:GROUP].rearrange("g i j -> i g j"))
            wt = wpool.tile([H, nmm * HO], bf16)
            nc.sync.dma_start(out=wt[:, :3 * HO], in_=wb[:, :3 * HO])
            nc.scalar.dma_start(out=wt[:, 3 * HO:], in_=wb[:, 3 * HO:])
            xt1 = xpool.tile([H, GROUP, W], f32, tag="x1")
            nc.scalar.dma_start(
                out=xt1[:, :, :],
                in_=xs[GROUP:2 * GROUP].rearrange("g i j -> i g j"))
            xtb = xpool.tile([H, IMG_PER_CORE - 2 * GROUP, W], f32, tag="xb")
            nc.scalar.dma_start(
                out=xtb[:, :, :],
                in_=xs[2 * GROUP:].rearrange("g i j -> i g j"))

            # bias tiles for the activation immediates
            kb = []
            for i, k in enumerate(knots):
                kt = cpool.tile([H, 1], f32, tag=f"kb{i}")
                nc.gpsimd.memset(kt[:, :], -float(k))
                kb.append(kt)
            bt0 = cpool.tile([H, 1], f32, tag="bias")
            nc.gpsimd.memset(bt0[:, :], float(bias))

            for g in range(N_GROUPS):
                xt = (xt0, xt1, xtb[:, :GROUP, :], xtb[:, GROUP:, :])[g]

                def zt(nm):
                    return zpool.tile([H, GROUP, W], bf16, tag=nm, name=nm)

                z1, z2, z3 = zt("z1"), zt("z2"), zt("z3")
                nc.vector.tensor_copy(out=z1[:], in_=xt[:])
                nc.vector.tensor_mul(out=z2[:], in0=z1[:], in1=z1[:])
                nc.vector.tensor_mul(out=z3[:], in0=z2[:], in1=z1[:])
                channels = [z1, z2, z3]
                for i, k in enumerate(knots):
                    u2 = zt(f"u2_{i}")
                    bi = zt(f"b_{i}")
                    z4 = zt(f"z4_{i}")
                    nc.scalar.activation(u2[:], xt[:], AF.Square, bias=kb[i][:, :])
                    nc.vector.tensor_scalar(
                        out=bi[:], in0=z1[:], scalar1=-k, scalar2=0.0,
                        op0=ALU.add, op1=ALU.max)
                    nc.vector.tensor_mul(out=z4[:], in0=bi[:], in1=u2[:])
                    channels.append(z4)

                pt = pp.tile([HO, GROUP, WO], f32, tag="acc")
                for tt in range(nmm):
                    ch, dx = divmod(tt, 3)
                    nc.tensor.matmul(
                        pt[:, :, :],
                        wt[:, tt * HO:(tt + 1) * HO],
                        channels[ch][:, :, dx:dx + WO],
                        start=(tt == 0),
                        stop=(tt == nmm - 1),
                    )

                ot = opool.tile([HO, GROUP, WO], bf16, tag=f"o{g}")
                nc.scalar.activation(
                    ot[:, :, :], pt[:], AF.Identity, bias=bt0[:HO, :])
                nc.sync.dma_start(out=out[g], in_=ot[:, :, :])
    return nc


def _split_multiwaits(bir_json_bytes):
    """This toolchain's walrus accepts at most ONE sync-wait per instruction,
    while Tile attaches several (up to 11 on the tail drain).  Rewrite the BIR:
    move all but the last wait of each instruction onto injected same-engine
    NoOps placed immediately before it (engine streams execute in block order,
    so waiting earlier on the same engine is equivalent).  The waits are
    sorted by the program position of the last updater of each semaphore, so
    the already-satisfied waits retire while the final DMA still runs instead
    of serializing after it.  Also drops the unused builtin const-AP memsets.
    """
    import json
    m = json.loads(bir_json_bytes)

    # program position of the last instruction updating each semaphore id
    last_upd = {}
    pos = 0
    for fn in m["functions"]:
        for bb in fn["blocks"]:
            for ins in bb["instructions"]:
                pos += 1
                for u in (ins.get("sync_info") or {}).get("on_update") or []:
                    if "id" in u:
                        last_upd[u["id"]] = pos

    n = 0
    for fn in m["functions"]:
        for bb in fn["blocks"]:
            is_main = bb.get("name") == "main"
            new = []
            for ins in bb["instructions"]:
                if (ins["opcode"] == "Memset"
                        and str((ins.get("outs") or [{}])[0].get("memref", ""))
                        .startswith("const-")):
                    continue
                # The init barrier only ordered the const memsets (stripped
                # above) against their users; drop it too.
                if is_main and ins["opcode"] in ("EventSemaphore", "Drain"):
                    continue
                si = ins.get("sync_info")
                waits = (si or {}).get("on_wait") or []
                if len(waits) > 1:
                    waits = sorted(
                        waits, key=lambda w: last_upd.get(w.get("id"), -1))
                    for w in waits[:-1]:
                        n += 1
                        new.append({
                            "debug": ins.get("debug", 0),
                            "engine": ins["engine"],
                            "ins": [], "outs": [],
                            "name": f"mwsplit-{n}",
                            "opcode": "NoOp",
                            "sync_info": {"on_update": [], "on_wait": [w]},
                        })
                    si["on_wait"] = [waits[-1]]
                new.append(ins)
            bb["instructions"] = new
    return json.dumps(m).encode()


def _get_nc(knots, bias):
    key = (tuple(np.round(knots, 6)), round(float(bias), 9))
    if key not in _NC_CACHE:
        nc = _build_nc(list(knots), float(bias))
        orig = type(nc).to_json_bytes
        nc.to_json_bytes = lambda *a, **k: _split_multiwaits(orig(nc, *a, **k))
        _NC_CACHE[key] = nc
    return _NC_CACHE[key]


def kernel(x, base_weight, spline_weight, spline_scaler, grid, _bench=None):
    from concourse.bass_utils import run_bass_kernel_spmd

    x = np.ascontiguousarray(np.asarray(x, dtype=np.float32))
    base_weight = np.asarray(base_weight, dtype=np.float32)
    spline_weight = np.asarray(spline_weight, dtype=np.float32)
    spline_scaler = np.asarray(spline_scaler, dtype=np.float32)
    grid = np.asarray(grid, dtype=np.float32)

    knots, Wc, bias = _host_fit(base_weight, spline_weight, spline_scaler, grid, x)
    wbm = np.ascontiguousarray(_banded_lhsT(Wc))

    xf = x.reshape(B * C, H, W)
    in_maps = [
        {"xs": np.ascontiguousarray(xf[k * IMG_PER_CORE:(k + 1) * IMG_PER_CORE]),
         "wb": wbm}
        for k in range(N_CORES)
    ]

    nc = _get_nc(knots, bias)
    kwargs = dict(_bench or {})
    res = run_bass_kernel_spmd(nc, in_maps, list(range(N_CORES)), **kwargs)
    if _bench is not None and isinstance(_bench, dict):
        _bench["results"] = res

    outs = []
    for k in range(N_CORES):
        o = np.asarray(res.results[k]["out"]).astype(np.float32)  # [NG, HO, G, WO]
        outs.append(o.transpose(0, 2, 1, 3).reshape(IMG_PER_CORE, HO, WO))
    full = np.concatenate(outs, axis=0).reshape(B, C, HO, WO)
    return np.ascontiguousarray(full.astype(np.float32))


# revision 20
# speedup vs baseline: 1.0572x; 1.0572x over previous
"""KAN 3x3 convolution (single KANLinear shared across channels) on 8 TRN2 cores.

Math: for x in [0,1) on the fixed uniform spline grid, every per-feature
function g_f(t) = spline_f(t) + base_w_f*silu(t) is smooth with (at most) C^2
breaks at the interior knots 0.2 / 0.6.  We fit (host-side, vs the actual
runtime weights) a truncated-power basis {1, t, t^2, t^3, relu(t-k_i)^3} with
an adaptively chosen knot set: [] -> 3 channels, [k*] -> 4, [0.2,0.6] -> 5
(exact).  The smallest basis whose predicted output error (measured on patches
sampled from the real input) is < TH is used, so the whole KAN conv collapses
to   out = bias + conv3x3(W, [x, x^2, x^3, relu(x-k_i)^3...]).

On-device per core: pointwise channel build (DVE/ACT, bf16) + NCH*3 banded
matmuls on TensorE (contraction over image rows; bands encode dy taps,
free-dim shifts encode dx), PSUM-accumulated per 4-image group, extracted
with the bias add fused into the ScalarE copy (bf16 out, SBUF-native layout;
the host undoes the group interleave).
"""

import numpy as np
import ml_dtypes

B, C, H, W = 16, 8, 128, 128
KERNEL = 3
HO = WO = H - KERNEL + 1  # 126
SPLINE_ORDER = 3
N_CORES = 8
IMG_PER_CORE = (B * C) // N_CORES  # 16
GROUP = 4                          # images per matmul group
N_GROUPS = IMG_PER_CORE // GROUP   # 4
BF16 = ml_dtypes.bfloat16
FIT_TH = 1.2e-2                    # predicted rel-err budget for basis choice

_NC_CACHE = {}


def _bsplines_np(t, grid):
    """Port of reference b_splines in numpy float64. t: (N,), grid: (F, G)."""
    F = grid.shape[0]
    x = np.tile(t[:, None], (1, F))[..., None]       # (N, F, 1)
    g = grid[None, :, :]                             # (1, F, G)
    bases = ((x >= g[:, :, :-1]) & (x < g[:, :, 1:])).astype(np.float64)
    for k in range(1, SPLINE_ORDER + 1):
        bases = ((x - g[:, :, : -(k + 1)]) / (g[:, :, k:-1] - g[:, :, : -(k + 1)])
                 * bases[:, :, :-1]
                 + (g[:, :, k + 1:] - x) / (g[:, :, k + 1:] - g[:, :, 1:-k])
                 * bases[:, :, 1:])
    return bases                                     # (N, F, G - order - 1)


def _host_fit(base_weight, spline_weight, spline_scaler, grid, x):
    """Pick the smallest truncated-power basis representing all 9 per-feature
    functions g_f within FIT_TH of the (sampled) output RMS.

    Returns (knots list, Wc [nch, 9] fp64 channel weights, bias fp64)."""
    c = (spline_weight[0].astype(np.float64)
         * spline_scaler[0].astype(np.float64)[:, None])          # (9, 8)
    t = np.linspace(0.0, 1.0, 4097, endpoint=False) + 1.0 / 8194.0
    bases = _bsplines_np(t, grid.astype(np.float64))              # (N, 9, 8)
    g = np.einsum("nfj,fj->nf", bases, c)                         # (N, 9)
    g += base_weight[0].astype(np.float64)[None, :] * (t / (1.0 + np.exp(-t)))[:, None]

    # output RMS estimate from patches of the actual input
    rng = np.random.RandomState(0)
    xf = x.reshape(B * C, H, W).astype(np.float64)
    ii = rng.randint(0, B * C, 20000)
    yy = rng.randint(0, HO, 20000)
    xx = rng.randint(0, WO, 20000)
    acc = np.zeros(20000)
    f = 0
    for dy in range(3):
        for dx in range(3):
            v = xf[ii, yy + dy, xx + dx]
            acc += np.interp(v, t, g[:, f])
            f += 1
    out_rms = max(np.sqrt((acc ** 2).mean()), 1e-30)

    def basis(knots):
        cols = [np.ones_like(t), t, t * t, t ** 3]
        cols += [np.maximum(t - k, 0.0) ** 3 for k in knots]
        return np.stack(cols, axis=-1)

    def fit(knots):
        A = basis(knots)
        coef, _, _, _ = np.linalg.lstsq(A, g, rcond=None)
        resid = A @ coef - g
        rel = np.sqrt((resid ** 2).mean(0).sum()) / out_rms
        return coef, rel

    cands = [[]]
    best_k, best_rel = None, None
    for k in np.linspace(0.05, 0.95, 91):
        _, rel = fit([k])
        if best_rel is None or rel < best_rel:
            best_k, best_rel = float(k), rel
    cands.append([best_k])
    cands.append([0.2, 0.6])
    for knots in cands:
        coef, rel = fit(knots)
        if rel < FIT_TH or knots == [0.2, 0.6]:
            Wc = coef[1:].copy()                  # (nch, 9): t, t^2, t^3, hinges
            bias = coef[0].sum()
            return knots, Wc, bias
    raise AssertionError("unreachable")


def _banded_lhsT(Wc):
    """[128, NCH*3*126] bf16: per (ch, dx) a banded [128,126] with
    W[ch, dy, dx] on diagonals (row i+dy, col i)."""
    nch = Wc.shape[0]
    Wk = Wc.reshape(nch, 3, 3)            # (ch, dy, dx)
    out = np.zeros((H, nch * 3 * HO), dtype=np.float64)
    ii = np.arange(HO)
    for ch in range(nch):
        for dx in range(3):
            tt = ch * 3 + dx
            for dy in range(3):
                out[ii + dy, tt * HO + ii] = Wk[ch, dy, dx]
    return out.astype(BF16)


def _build_nc(knots, bias):
    import concourse.bass as bass
    import concourse.mybir as mybir
    from concourse.tile import TileContext

    f32 = mybir.dt.float32
    bf16 = mybir.dt.bfloat16
    AF = mybir.ActivationFunctionType
    ALU = mybir.AluOpType
    nch = 3 + len(knots)
    nmm = nch * 3

    nc = bass.Bass()
    xs = nc.declare_dram_parameter("xs", [IMG_PER_CORE, H, W], f32, isOutput=False)
    wb = nc.declare_dram_parameter("wb", [H, nmm * HO], bf16, isOutput=False)
    out = nc.declare_dram_parameter(
        "out", [N_GROUPS, HO, GROUP, WO], bf16, isOutput=True)

    with TileContext(nc) as tc:
        with tc.tile_pool(name="wpool", bufs=1) as wpool, \
             tc.tile_pool(name="xpool", bufs=1) as xpool, \
             tc.tile_pool(name="zpool", bufs=2) as zpool, \
             tc.tile_pool(name="opool", bufs=1) as opool, \
             tc.tile_pool(name="cpool", bufs=1) as cpool, \
             tc.tile_pool(name="psum", bufs=4, space="PSUM") as pp, \
             tc.tile_pool(name="psumj", bufs=1, space="PSUM") as ppj:
            # --- PE warm-up: junk matmuls during the DMA fill keep the HAM
            # activity window busy so the real matmuls run at 2.4 GHz.  Any
            # PE idle gap resets the HAM busy streak, so the dummies must
            # hand off seamlessly to the first real matmul.
            junk = cpool.tile([H, 512], bf16, tag="junk")
            nc.gpsimd.memset(junk[:, :], 0.0)
            ptj = ppj.tile([H, 512], f32, tag="jacc")
            for _ in range(5):
                nc.tensor.matmul(ptj[:, :], junk[:, :H], junk[:, :],
                                 start=True, stop=True)
            for _ in range(9):
                nc.tensor.matmul(ptj[:, :128], junk[:, :H], junk[:, :128],
                                 start=True, stop=True)

            # --- input DMAs.  group-0 images go first and own the SDMA
            # engines (the bulk-x DMA is WAW-delayed behind them below),
            # split across the sync and vector DGE rings so their triggers
            # issue in parallel; weight bands ride the scalar-engine ring.
            # The scalar-engine DGE ring generates descriptors in FIFO order,
            # so queueing [xt0b, wtA, wtB, xt1, xt23] there delivers each
            # piece in priority order with per-piece completion semaphores,
            # while the sync ring carries xt0a in parallel.
            # wt alone on the sync ring (small, gates the first matmul); the
            # x parts FIFO-chain on the scalar ring in group order, so each
            # group's completion semaphore fires incrementally.
            wt = wpool.tile([H, nmm * HO], bf16)
            nc.sync.dma_start(out=wt[:, :], in_=wb[:, :])
            xt0 = xpool.tile([H, GROUP, W], f32, tag="x0")
            nc.scalar.dma_start(
                out=xt0[:, :, :],
                in_=xs[0:GROUP].rearrange("g i j -> i g j"))
            xt1 = xpool.tile([H, GROUP, W], f32, tag="x1")
            nc.scalar.dma_start(
                out=xt1[:, :, :],
                in_=xs[GROUP:2 * GROUP].rearrange("g i j -> i g j"))
            xtb = xpool.tile([H, IMG_PER_CORE - 2 * GROUP, W], f32, tag="xb")
            nc.scalar.dma_start(
                out=xtb[:, :, :],
                in_=xs[2 * GROUP:].rearrange("g i j -> i g j"))

            # bias tiles for the activation immediates
            kb = []
            for i, k in enumerate(knots):
                kt = cpool.tile([H, 1], f32, tag=f"kb{i}")
                nc.gpsimd.memset(kt[:, :], -float(k))
                kb.append(kt)
            bt0 = cpool.tile([H, 1], f32, tag="bias")
            nc.gpsimd.memset(bt0[:, :], float(bias))

            for g in range(N_GROUPS):
                xt = (xt0, xt1, xtb[:, :GROUP, :], xtb[:, GROUP:, :])[g]

                def zt(nm):
                    return zpool.tile([H, GROUP, W], bf16, tag=nm, name=nm)

                z1, z2, z3 = zt("z1"), zt("z2"), zt("z3")
                nc.vector.tensor_copy(out=z1[:], in_=xt[:])
                nc.vector.tensor_mul(out=z2[:], in0=z1[:], in1=z1[:])
                nc.vector.tensor_mul(out=z3[:], in0=z2[:], in1=z1[:])
                channels = [z1, z2, z3]
                for i, k in enumerate(knots):
                    u2 = zt(f"u2_{i}")
                    bi = zt(f"b_{i}")
                    z4 = zt(f"z4_{i}")
                    nc.scalar.activation(u2[:], xt[:], AF.Square, bias=kb[i][:, :])
                    nc.vector.tensor_scalar(
                        out=bi[:], in0=z1[:], scalar1=-k, scalar2=0.0,
                        op0=ALU.add, op1=ALU.max)
                    nc.vector.tensor_mul(out=z4[:], in0=bi[:], in1=u2[:])
                    channels.append(z4)

                pt = pp.tile([HO, GROUP, WO], f32, tag="acc")
                for tt in range(nmm):
                    ch, dx = divmod(tt, 3)
                    nc.tensor.matmul(
                        pt[:, :, :],
                        wt[:, tt * HO:(tt + 1) * HO],
                        channels[ch][:, :, dx:dx + WO],
                        start=(tt == 0),
                        stop=(tt == nmm - 1),
                    )

                ot = opool.tile([HO, GROUP, WO], bf16, tag=f"o{g}")
                nc.scalar.activation(
                    ot[:, :, :], pt[:], AF.Identity, bias=bt0[:HO, :])
                nc.sync.dma_start(out=out[g], in_=ot[:, :, :])
    return nc


def _split_multiwaits(bir_json_bytes):
    """This toolchain's walrus accepts at most ONE sync-wait per instruction,
    while Tile attaches several (up to 11 on the tail drain).  Rewrite the BIR:
    move all but the last wait of each instruction onto injected same-engine
    NoOps placed immediately before it (engine streams execute in block order,
    so waiting earlier on the same engine is equivalent).  The waits are
    sorted by the program position of the last updater of each semaphore, so
    the already-satisfied waits retire while the final DMA still runs instead
    of serializing after it.  Also drops the unused builtin const-AP memsets.
    """
    import json
    m = json.loads(bir_json_bytes)

    # program position of the last instruction updating each semaphore id
    last_upd = {}
    pos = 0
    for fn in m["functions"]:
        for bb in fn["blocks"]:
            for ins in bb["instructions"]:
                pos += 1
                for u in (ins.get("sync_info") or {}).get("on_update") or []:
                    if "id" in u:
                        last_upd[u["id"]] = pos

    n = 0
    for fn in m["functions"]:
        for bb in fn["blocks"]:
            is_main = bb.get("name") == "main"
            new = []
            for ins in bb["instructions"]:
                if (ins["opcode"] == "Memset"
                        and str((ins.get("outs") or [{}])[0].get("memref", ""))
                        .startswith("const-")):
                    continue
                # The init barrier only ordered the const memsets (stripped
                # above) against their users; drop it too.
                if is_main and ins["opcode"] in ("EventSemaphore", "Drain"):
                    continue
                si = ins.get("sync_info")
                waits = (si or {}).get("on_wait") or []
                if len(waits) > 1:
                    waits = sorted(
                        waits, key=lambda w: last_upd.get(w.get("id"), -1))
                    for w in waits[:-1]:
                        n += 1
                        new.append({
                            "debug": ins.get("debug", 0),
                            "engine": ins["engine"],
                            "ins": [], "outs": [],
                            "name": f"mwsplit-{n}",
                            "opcode": "NoOp",
                            "sync_info": {"on_update": [], "on_wait": [w]},
                        })
                    si["on_wait"] = [waits[-1]]
                new.append(ins)
            bb["instructions"] = new
    return json.dumps(m).encode()


def _get_nc(knots, bias):
    key = (tuple(np.round(knots, 6)), round(float(bias), 9))
    if key not in _NC_CACHE:
        nc = _build_nc(list(knots), float(bias))
        orig = type(nc).to_json_bytes
        nc.to_json_bytes = lambda *a, **k: _split_multiwaits(orig(nc, *a, **k))
        _NC_CACHE[key] = nc
    return _NC_CACHE[key]


def kernel(x, base_weight, spline_weight, spline_scaler, grid, _bench=None):
    from concourse.bass_utils import run_bass_kernel_spmd

    x = np.ascontiguousarray(np.asarray(x, dtype=np.float32))
    base_weight = np.asarray(base_weight, dtype=np.float32)
    spline_weight = np.asarray(spline_weight, dtype=np.float32)
    spline_scaler = np.asarray(spline_scaler, dtype=np.float32)
    grid = np.asarray(grid, dtype=np.float32)

    knots, Wc, bias = _host_fit(base_weight, spline_weight, spline_scaler, grid, x)
    wbm = np.ascontiguousarray(_banded_lhsT(Wc))

    xf = x.reshape(B * C, H, W)
    in_maps = [
        {"xs": np.ascontiguousarray(xf[k * IMG_PER_CORE:(k + 1) * IMG_PER_CORE]),
         "wb": wbm}
        for k in range(N_CORES)
    ]

    nc = _get_nc(knots, bias)
    kwargs = dict(_bench or {})
    res = run_bass_kernel_spmd(nc, in_maps, list(range(N_CORES)), **kwargs)
    if _bench is not None and isinstance(_bench, dict):
        _bench["results"] = res

    outs = []
    for k in range(N_CORES):
        o = np.asarray(res.results[k]["out"]).astype(np.float32)  # [NG, HO, G, WO]
        outs.append(o.transpose(0, 2, 1, 3).reshape(IMG_PER_CORE, HO, WO))
    full = np.concatenate(outs, axis=0).reshape(B, C, HO, WO)
    return np.ascontiguousarray(full.astype(np.float32))
